# revision 19
# baseline (speedup 1.0000x reference)
"""Trainium2 Bass kernel for the NeRF coordinate-chain problem.

Reference semantics: flat_dihedrals [1048576, 3] is (row-major) reinterpreted
as phi[K=6144, B=512]; each of the 512 columns is an independent serial NeRF
chain of K rigid-body extension steps, with bond-geometry constants cycling as
d = (2k + b) mod 3.

Reformulation (v2): the per-step update is an affine (SE3) composition
    T_k = T_{k-1} o A_k,  A_k = [[Rx(phi_k) Rz(theta_d), p_k],[0,1]],
    p_k = r_d * col0(Rx Rz),  T_0 = Identity,  coord_k = translation(T_k).
Because rotations distribute over sums, coord_k = sum_{k'<=k} v_k' with
v_k' = Rprefix[k'-1] @ p_k' -- a pure cumulative sum of rotated step
increments.  Pipeline per core (64 batch columns, partitions p = c + 64*h
carrying two k-halves):
  L0 (fp32, DVE + Scalar): 24-step serial chain over NQ=128 blocks/half in a
    q-innermost layout R[p, row, col, q]; trig tables from the Act engine;
    w_s = r * col0(R) extracted on the Act engine into fp16.
  Block scan (fp16, DVE): Sklansky prefix over the 128 block rotations per
    half + cross-half fixup via a tiny SBUF DMA.
  Apply (fp16, DVE): v = ShR @ w as 9 fused-broadcast multiplies; block-local
    serial cumsum over s; per-block prefix via the hardware scan; broadcast
    block-offset add.
  Output: PE transposes (fp16 -> PSUM fp32 upconvert) into k-major staging
    tiles, contiguous DMAs to DRAM.

Sharding: batch columns split across 8 cores (64 columns/core).
"""

import numpy as np

L_STEPS = 2048
B_FULL = 512
NUM_CORES = 8
BC = B_FULL // NUM_CORES          # batch columns per core
K = 3 * L_STEPS                   # chain length = 6144
S0 = 12                           # L0 block size (multiple of 3)
NQ = 256                          # blocks per partition-half
NB2 = 2                           # partition halves (k = 3072*h + 24*q + s)
KTW = 96                          # k-tile width for output transposes (4 blocks)
NKT = (S0 * NQ) // KTW            # 32 k-tiles per half

_BL = np.array([145.801, 152.326, 132.868], dtype=np.float32)
_BA = np.array([2.124, 1.941, 2.028], dtype=np.float32)
_CT = np.cos(np.pi - _BA).astype(np.float32)
_ST = np.sin(np.pi - _BA).astype(np.float32)

_CACHE = {}


def _build_program(reps: int = 1, only: str = ""):
    """Build the program.  `only` repeats a single phase inside the reps loop
    ("l0" | "scan" | "apply") for phase-level HW timing; "" = full kernel
    repeated per rep."""
    import concourse.bass as bass
    import concourse.tile as tile
    from concourse import bacc, masks, mybir
    from concourse._compat import axon_active

    f32 = mybir.dt.float32
    f16 = mybir.dt.float16
    Al = mybir.AluOpType
    Act = mybir.ActivationFunctionType

    nc = bacc.Bacc(
        "TRN2",
        target_bir_lowering=False,
        debug=not axon_active(),
        enable_asserts=False,
        num_devices=NUM_CORES,
    )
    phi_d = nc.dram_tensor("phi", [128, S0, NQ], f32, kind="ExternalInput").ap()
    mtab_d = nc.dram_tensor("mtab", [128, S0, 3], f32, kind="ExternalInput").ap()
    out_d = nc.dram_tensor("out", [K, BC, 3], f32, kind="ExternalOutput").ap()

    with tile.TileContext(nc) as tc:
        with tc.tile_pool(name="main", bufs=1) as pool:
            S = {}

            def ph_l0():
                # ---------------- load inputs + trig tables ----------------
                mtab = S["mtab"] = pool.tile([128, S0, 3], f32, tag="mtab", name="mtab")
                nc.sync.dma_start(mtab[:], mtab_d[:])
                phi = pool.tile([128, S0, NQ], f32, tag="phi", name="phi")
                pih = pool.tile([128, 1], f32, tag="pih", name="pih")
                zero = pool.tile([128, 1], f32, tag="zero", name="zero")
                nc.vector.memset(pih[:], float(np.pi / 2))
                nc.vector.memset(zero[:], 0.0)
                cosT = pool.tile([128, S0, NQ], f32, tag="cosT", name="cosT")
                snsT = pool.tile([128, S0, 2, NQ], f32, tag="snsT", name="snsT")
                absT = pool.tile([128, S0, NQ], f32, tag="absT", name="absT")
                SCH = 4  # s-chunk for trig build (overlaps with L0 start)
                for ch in range(S0 // SCH):
                    sl = slice(ch * SCH, (ch + 1) * SCH)
                    nc.sync.dma_start(phi[:, sl, :], phi_d[:, sl, :])
                    nc.scalar.activation(absT[:, sl, :], phi[:, sl, :], Act.Abs, bias=zero[:, :])
                    nc.scalar.activation(cosT[:, sl, :], absT[:, sl, :], Act.Sin, bias=pih[:, :], scale=-1.0)
                    nc.scalar.activation(snsT[:, sl, 0, :], phi[:, sl, :], Act.Sin, bias=zero[:, :])
                    nc.scalar.activation(snsT[:, sl, 1, :], phi[:, sl, :], Act.Sin, bias=zero[:, :], scale=-1.0)

                # ---------------- L0 serial chain (fp32) ----------------
                # R[p, col k, row i, q] (col-outer so every slice is one
                # contiguous run); cols evolve under Rx (cols 1,2) and
                # Rz (cols 0,1); w_s = r * col0 extracted on the Act engine.
                R = S["R"] = pool.tile([128, 3, 3, NQ], f32, tag="R", name="R")
                w = S["w"] = pool.tile([128, 3, S0, NQ], f16, tag="w", name="w")
                m12 = pool.tile([128, 2, 3, NQ], f32, tag="m12", name="m12")
                m34 = pool.tile([128, 2, 3, NQ], f32, tag="m34", name="m34")
                tb01 = pool.tile([128, 2, 3, NQ], f32, tag="tb01", name="tb01")
                nc.vector.memset(R[:], 0.0)
                for i in range(3):
                    nc.vector.memset(R[:, i, i, :], 1.0)

                R12 = R[:, 1:3, :, :]
                R21 = R12[:, ::-1, :, :]
                R01 = R[:, 0:2, :, :]
                for s in range(S0):
                    cosb = cosT[:, s : s + 1, :].unsqueeze(2).broadcast_to([128, 2, 3, NQ])
                    snsb = snsT[:, s, :, :].unsqueeze(2).broadcast_to([128, 2, 3, NQ])
                    # Rx: (c1,c2) <- (cp*c1 + sp*c2, cp*c2 - sp*c1)
                    nc.vector.tensor_tensor(m12[:], R12, cosb, op=Al.mult)
                    nc.vector.tensor_tensor(m34[:], R21, snsb, op=Al.mult)
                    nc.vector.tensor_tensor(R12, m12[:], m34[:], op=Al.add)
                    # Rz: (c0,c1) <- (ct*c0 + st*c1, ct*c1 - st*c0)
                    # tb01 = [st*c0, st*c1]; consumers cross-index the slices
                    nc.vector.tensor_scalar(tb01[:], R01, mtab[:, s, 1:2], None, op0=Al.mult)
                    nc.vector.scalar_tensor_tensor(
                        R[:, 0], R[:, 0], mtab[:, s, 0:1], tb01[:, 1],
                        op0=Al.mult, op1=Al.add,
                    )
                    nc.vector.scalar_tensor_tensor(
                        R[:, 1], R[:, 1], mtab[:, s, 0:1], tb01[:, 0],
                        op0=Al.mult, op1=Al.subtract,
                    )
                    # w_s = r * col0(R)  (Act engine, fp16 out)
                    nc.scalar.mul(w[:, :, s, :], R[:, 0], mtab[:, s, 2:3])

            def ph_scan():
                # ------------- block-prefix rotations (fp16) -------------
                R = S["R"]
                Pq = pool.tile([128, NQ, 9], f16, tag="Pq", name="Pq")
                nc.vector.tensor_copy(
                    Pq[:].rearrange("p q (k i) -> p q k i", k=3),
                    R[:].rearrange("p k i q -> p q k i"),
                )
                ma = pool.tile([128, NQ // 2, 9], f16, tag="ma", name="ma")
                mb = pool.tile([128, NQ // 2, 9], f16, tag="mb", name="mb")
                mc = pool.tile([128, NQ // 2, 9], f16, tag="mc", name="mc")

                def compose(dst, aR, bR, g):
                    """dst = Ra @ Rb (column-major entries e = 3k + i)."""
                    for kk in range(3):
                        colk = aR[:, :, 3 * kk : 3 * kk + 3].unsqueeze(2).broadcast_to([128, g, 3, 3])
                        rowk = bR[:, :, kk::3].unsqueeze(3).broadcast_to([128, g, 3, 3])
                        mv = (ma, mb, mc)[kk][:, 0:g].rearrange("p g (f t) -> p g f t", f=3)
                        nc.vector.tensor_tensor(mv, colk, rowk, op=Al.mult)
                    nc.vector.tensor_tensor(ma[:, 0:g], ma[:, 0:g], mb[:, 0:g], op=Al.add)
                    nc.vector.tensor_tensor(dst, ma[:, 0:g], mc[:, 0:g], op=Al.add)

                # Brent-Kung in-place inclusive scan over q (per half)
                d = 1
                while d < NQ:
                    n = NQ // (2 * d)
                    a = Pq[:].rearrange("p (m j) e -> p m j e", j=2 * d)[:, :, d - 1, :]
                    b = Pq[:].rearrange("p (m j) e -> p m j e", j=2 * d)[:, :, 2 * d - 1, :]
                    compose(b, a, b, n)
                    d *= 2
                d = NQ // 4
                while d >= 1:
                    n = NQ // (2 * d) - 1
                    a = Pq[:].rearrange("p (m j) e -> p m j e", j=2 * d)[:, 0:n, 2 * d - 1, :]
                    b = Pq[:].rearrange("p (m j) e -> p m j e", j=2 * d)[:, 1 : n + 1, d - 1, :]
                    compose(b, a, b, n)
                    d //= 2

                # cross-half: lower-half total -> upper partitions
                stgRh = pool.tile([128, 1, 9], f16, tag="stgRh", name="stgRh")
                nc.sync.dma_start(stgRh[64:128, :, :], Pq[0:64, NQ - 1 : NQ, :])
                stgR = pool.tile([128, 1, 9], f32, tag="stgR", name="stgR")
                nc.vector.tensor_copy(stgR[64:128, :, :], stgRh[64:128, :, :])

                # ShR (exclusive prefix, matvec layout [p, i, k, q], fp16)
                ShR = S["ShR"] = pool.tile([128, 3, 3, NQ], f16, tag="ShR", name="ShR")
                nc.vector.memset(ShR[:, :, :, 0:1], 0.0)
                for i in range(3):
                    nc.vector.memset(ShR[0:64, i, i, 0:1], 1.0)
                nc.vector.tensor_copy(
                    ShR[0:64, :, :, 1:NQ],
                    Pq[0:64, 0 : NQ - 1, :].rearrange("p q (k i) -> p i k q", k=3),
                )
                # upper half: ShR_up[q] = stgR o Pshift_up[q]
                tmp = pool.tile([128, 3, 3, NQ], f16, tag="tmpU", name="tmpU")
                nc.vector.memset(tmp[64:128, :, :, 0:1], 0.0)
                for i in range(3):
                    nc.vector.memset(tmp[64:128, i, i, 0:1], 1.0)
                nc.vector.tensor_copy(
                    tmp[64:128, :, :, 1:NQ],
                    Pq[64:128, 0 : NQ - 1, :].rearrange("p q (k i) -> p i k q", k=3),
                )
                sa = pool.tile([128, 3, NQ], f16, tag="sa", name="sa")
                sb = pool.tile([128, 3, NQ], f16, tag="sb", name="sb")
                for i in range(3):
                    nc.vector.tensor_scalar(
                        sa[64:128], tmp[64:128, 0, :, :], stgR[64:128, 0, 3 * 0 + i : 3 * 0 + i + 1],
                        None, op0=Al.mult,
                    )
                    nc.vector.tensor_scalar(
                        sb[64:128], tmp[64:128, 1, :, :], stgR[64:128, 0, 3 * 1 + i : 3 * 1 + i + 1],
                        None, op0=Al.mult,
                    )
                    nc.vector.tensor_tensor(sa[64:128], sa[64:128], sb[64:128], op=Al.add)
                    nc.vector.tensor_scalar(
                        sb[64:128], tmp[64:128, 2, :, :], stgR[64:128, 0, 3 * 2 + i : 3 * 2 + i + 1],
                        None, op0=Al.mult,
                    )
                    nc.vector.tensor_tensor(ShR[64:128, i, :, :], sa[64:128], sb[64:128], op=Al.add)

            def ph_apply():
                # ---- v = ShR @ w ; cumsum in s; block-prefix; offset add ----
                w, ShR = S["w"], S["ShR"]
                v = S["v"] = pool.tile([128, 3, S0, NQ], f16, tag="v", name="v")
                u1 = pool.tile([128, S0, NQ], f16, tag="u1", name="u1")
                u2 = pool.tile([128, S0, NQ], f16, tag="u2", name="u2")
                for i in range(3):
                    sh = lambda j: ShR[:, i, j, :].unsqueeze(1).broadcast_to([128, S0, NQ])
                    nc.vector.tensor_tensor(v[:, i], w[:, 0], sh(0), op=Al.mult)
                    nc.vector.tensor_tensor(u1[:], w[:, 1], sh(1), op=Al.mult)
                    nc.vector.tensor_tensor(u2[:], w[:, 2], sh(2), op=Al.mult)
                    nc.vector.tensor_tensor(v[:, i], v[:, i], u1[:], op=Al.add)
                    nc.vector.tensor_tensor(v[:, i], v[:, i], u2[:], op=Al.add)
                # block-local cumsum over s (in place, GpSimd -- off the
                # DVE critical path so the next rep's L0 can start)
                for s in range(1, S0):
                    nc.gpsimd.tensor_tensor(
                        v[:, :, s, :], v[:, :, s, :], v[:, :, s - 1, :], op=Al.add
                    )
                # block prefix: TI = inclusive scan_q of block totals (fp32)
                vz = pool.tile([128, NQ], f32, tag="vz", name="vz")
                nc.vector.memset(vz[:], 0.0)
                TI = pool.tile([128, 3, NQ], f32, tag="TI", name="TI")
                for i in range(3):
                    nc.vector.tensor_tensor_scan(
                        TI[:, i], v[:, i, S0 - 1, :], vz[:], 0.0, op0=Al.add, op1=Al.add
                    )
                # cross-half translation total
                stgT = pool.tile([128, 3, 1], f32, tag="stgT", name="stgT")
                nc.sync.dma_start(stgT[64:128], TI[0:64, :, NQ - 1 : NQ])
                Tpre = pool.tile([128, 3, NQ], f16, tag="Tpre", name="Tpre")
                nc.vector.memset(Tpre[:, :, 0:1], 0.0)
                nc.vector.tensor_copy(Tpre[0:64, :, 1:NQ], TI[0:64, :, 0 : NQ - 1])
                nc.vector.scalar_tensor_tensor(
                    Tpre[64:128, :, 1:NQ], TI[64:128, :, 0 : NQ - 1], 1.0,
                    stgT[64:128].broadcast_to([64, 3, NQ - 1]), op0=Al.mult, op1=Al.add,
                )
                nc.vector.tensor_copy(Tpre[64:128, :, 0:1], stgT[64:128])
                # coords = v + Tpre (broadcast over s) in place (GpSimd)
                nc.gpsimd.tensor_tensor(
                    v[:], v[:], Tpre[:].unsqueeze(2).broadcast_to([128, 3, S0, NQ]),
                    op=Al.add,
                )

                # ---------------- transpose + store ----------------
                # Transpose tile (i, s) = [p, q] directly from the natural
                # layout; PSUM rows become the 128 blocks q of step s, and the
                # DMA's per-partition k-stride (24 rows) scatters them while
                # each partition still writes one contiguous 768B run.
                ident = pool.tile([128, 128], f16, tag="ident", name="ident")
                masks.make_identity(nc, ident[:])
                out_dv = out_d.rearrange("(h q t) c i -> h t q c i", h=NB2, q=NQ)
                with tc.tile_pool(name="psum", bufs=6, space="PSUM") as psum:
                    for s in range(S0):
                        for qh in range(NQ // 128):
                            ql = slice(qh * 128, (qh + 1) * 128)
                            Bk = pool.tile([128, NB2, BC, 3], f32, tag="Bk",
                                           name=f"Bk{s}_{qh}", bufs=2)
                            for i in range(3):
                                pt = psum.tile([128, 128], f16, tag="pt", name="pt")
                                nc.tensor.transpose(pt[:], v[:, i, s, ql], ident[:])
                                ptv = pt[:].rearrange("p (h c) -> p h c", h=NB2)
                                nc.scalar.copy(Bk[:, :, :, i], ptv)
                            for h in range(NB2):
                                nc.sync.dma_start(out_dv[h, s, ql], Bk[:, h])

            def dummy_out():
                nc.sync.dma_start(out_d[0:1, :, :], S["R"][0:1, 0, 0, 0:BC].unsqueeze(2).broadcast_to([1, BC, 3]))

            if only == "":
                for _rep in range(reps):
                    ph_l0(); ph_scan(); ph_apply()
            elif only == "l0":
                for _rep in range(reps):
                    ph_l0()
                dummy_out()
            elif only == "scan":
                ph_l0()
                for _rep in range(reps):
                    ph_scan()
                dummy_out()
            elif only == "l0scan":
                for _rep in range(reps):
                    ph_l0(); ph_scan()
                dummy_out()
            elif only == "apply":
                ph_l0(); ph_scan()
                for _rep in range(reps):
                    ph_apply()
            else:
                raise ValueError(only)

    nc.compile()
    return nc


def _get_program(reps: int = 1):
    import os as _os
    only = _os.environ.get("KM_ONLY", "")
    key = ("nc", reps, only)
    if key not in _CACHE:
        _CACHE[key] = _build_program(reps, only)
    return _CACHE[key]


def _make_mtab(core: int) -> np.ndarray:
    p = np.arange(128)
    c = p % 64
    bprime = 64 * core + c
    s = np.arange(S0)
    d = (2 * s[None, :] + bprime[:, None]) % 3
    mt = np.stack([_CT[d], _ST[d], np.broadcast_to(_BL[d], d.shape)], axis=-1)
    return np.ascontiguousarray(mt.astype(np.float32))


LAST_RUN = {}


def _get_runner(reps: int = 1):
    """Build (once) a cached jitted 8-core executable."""
    rkey = ("runner", reps)
    if rkey in _CACHE:
        return _CACHE[rkey]
    import jax
    from jax.sharding import Mesh, PartitionSpec
    from jax.experimental.shard_map import shard_map
    from concourse import bass2jax, mybir

    nc = _get_program(reps)
    bass2jax.install_neuronx_cc_hook()

    partition_name = nc.partition_id_tensor.name if nc.partition_id_tensor else None
    in_names, out_names, out_avals = [], [], []
    for alloc in nc.m.functions[0].allocations:
        if not isinstance(alloc, mybir.MemoryLocationSet):
            continue
        name = alloc.memorylocations[0].name
        if alloc.kind == "ExternalInput":
            if name != partition_name:
                in_names.append(name)
        elif alloc.kind == "ExternalOutput":
            out_names.append(name)
            out_avals.append(
                jax.core.ShapedArray(tuple(alloc.tensor_shape), mybir.dt.np(alloc.dtype))
            )
    n_params = len(in_names)
    all_names = list(in_names) + list(out_names)
    if partition_name is not None:
        all_names.append(partition_name)

    def _body(*args):
        operands = list(args)
        if partition_name is not None:
            operands.append(bass2jax.partition_id_tensor())
        outs = bass2jax._bass_exec_p.bind(
            *operands,
            out_avals=tuple(out_avals),
            in_names=tuple(all_names),
            out_names=tuple(out_names),
            lowering_input_output_aliases=(),
            sim_require_finite=True,
            sim_require_nnan=True,
            nc=nc,
        )
        return tuple(outs)

    devices = jax.devices()[:NUM_CORES]
    mesh = Mesh(np.asarray(devices), ("core",))
    nin = n_params + len(out_names)
    sharded = jax.jit(
        shard_map(
            _body,
            mesh=mesh,
            in_specs=(PartitionSpec("core"),) * nin,
            out_specs=(PartitionSpec("core"),) * len(out_names),
            check_rep=False,
        ),
        keep_unused=True,
    )
    _CACHE[rkey] = (sharded, in_names, out_names, mesh)
    return _CACHE[rkey]


def _prep_inputs(phi: np.ndarray):
    """phi [K, B_FULL] -> dict of concatenated per-core inputs.

    Device layout phi[p, s, q] with p = c + 64*h, global step
    k = 3072*h + 24*q + s, batch column b = 64*core + c."""
    cores = []
    phir = phi.reshape(NB2, NQ, S0, B_FULL)          # [h, q, s, b]
    for c in range(NUM_CORES):
        pc = phir[:, :, :, c * BC : (c + 1) * BC]    # [h, q, s, c]
        arr = pc.transpose(0, 3, 2, 1)               # [h, c, s, q]
        cores.append(np.ascontiguousarray(arr.reshape(128, S0, NQ)))
    phi_cat = np.concatenate(cores, axis=0)
    mtab_cat = np.concatenate([_make_mtab(c) for c in range(NUM_CORES)], axis=0)
    return {"phi": phi_cat, "mtab": mtab_cat}


def _run(inputs: dict) -> np.ndarray:
    sharded, in_names, out_names, _ = _get_runner()
    zeros = [np.zeros((NUM_CORES * K, BC, 3), np.float32)]
    out_arrs = sharded(*[inputs[n] for n in in_names], *zeros)
    out_cat = np.asarray(out_arrs[0])
    return out_cat


def kernel(flat_dihedrals: np.ndarray, batch_size) -> np.ndarray:
    B = int(batch_size)
    assert B == B_FULL and flat_dihedrals.shape == (K * B // 3, 3), (
        f"hardcoded for {(K * B_FULL // 3, 3)}, got {flat_dihedrals.shape}, B={B}"
    )
    phi = np.ascontiguousarray(np.asarray(flat_dihedrals, dtype=np.float32).reshape(K, B))
    out_cat = _run(_prep_inputs(phi))
    per_core = out_cat.reshape(NUM_CORES, K, BC, 3)
    out = np.empty((K, B, 3), dtype=np.float32)
    for core in range(NUM_CORES):
        out[:, core * BC : (core + 1) * BC, :] = per_core[core]
    return out


# revision 20
# speedup vs baseline: 1.0486x; 1.0486x over previous
"""Trainium2 Bass kernel for the NeRF coordinate-chain problem.

Reference semantics: flat_dihedrals [1048576, 3] is (row-major) reinterpreted
as phi[K=6144, B=512]; each of the 512 columns is an independent serial NeRF
chain of K rigid-body extension steps, with bond-geometry constants cycling as
d = (2k + b) mod 3.

Reformulation (v2): the per-step update is an affine (SE3) composition
    T_k = T_{k-1} o A_k,  A_k = [[Rx(phi_k) Rz(theta_d), p_k],[0,1]],
    p_k = r_d * col0(Rx Rz),  T_0 = Identity,  coord_k = translation(T_k).
Because rotations distribute over sums, coord_k = sum_{k'<=k} v_k' with
v_k' = Rprefix[k'-1] @ p_k' -- a pure cumulative sum of rotated step
increments.  Pipeline per core (64 batch columns, partitions p = c + 64*h
carrying two k-halves):
  L0 (fp32, DVE + Scalar): 24-step serial chain over NQ=128 blocks/half in a
    q-innermost layout R[p, row, col, q]; trig tables from the Act engine;
    w_s = r * col0(R) extracted on the Act engine into fp16.
  Block scan (fp16, DVE): Sklansky prefix over the 128 block rotations per
    half + cross-half fixup via a tiny SBUF DMA.
  Apply (fp16, DVE): v = ShR @ w as 9 fused-broadcast multiplies; block-local
    serial cumsum over s; per-block prefix via the hardware scan; broadcast
    block-offset add.
  Output: PE transposes (fp16 -> PSUM fp32 upconvert) into k-major staging
    tiles, contiguous DMAs to DRAM.

Sharding: batch columns split across 8 cores (64 columns/core).
"""

import numpy as np

L_STEPS = 2048
B_FULL = 512
NUM_CORES = 8
BC = B_FULL // NUM_CORES          # batch columns per core
K = 3 * L_STEPS                   # chain length = 6144
S0 = 12                           # L0 block size (multiple of 3)
NQ = 256                          # blocks per partition-half
NB2 = 2                           # partition halves (k = 3072*h + 24*q + s)
KTW = 96                          # k-tile width for output transposes (4 blocks)
NKT = (S0 * NQ) // KTW            # 32 k-tiles per half

_BL = np.array([145.801, 152.326, 132.868], dtype=np.float32)
_BA = np.array([2.124, 1.941, 2.028], dtype=np.float32)
_CT = np.cos(np.pi - _BA).astype(np.float32)
_ST = np.sin(np.pi - _BA).astype(np.float32)

_CACHE = {}


def _build_program(reps: int = 1, only: str = ""):
    """Build the program.  `only` repeats a single phase inside the reps loop
    ("l0" | "scan" | "apply") for phase-level HW timing; "" = full kernel
    repeated per rep."""
    import concourse.bass as bass
    import concourse.tile as tile
    from concourse import bacc, masks, mybir
    from concourse._compat import axon_active

    f32 = mybir.dt.float32
    f16 = mybir.dt.float16
    Al = mybir.AluOpType
    Act = mybir.ActivationFunctionType

    nc = bacc.Bacc(
        "TRN2",
        target_bir_lowering=False,
        debug=not axon_active(),
        enable_asserts=False,
        num_devices=NUM_CORES,
    )
    phi_d = nc.dram_tensor("phi", [128, S0, NQ], f32, kind="ExternalInput").ap()
    mtab_d = nc.dram_tensor("mtab", [128, S0, 3], f32, kind="ExternalInput").ap()
    out_d = nc.dram_tensor("out", [K, BC, 3], f32, kind="ExternalOutput").ap()

    with tile.TileContext(nc) as tc:
        with tc.tile_pool(name="main", bufs=1) as pool:
            S = {}

            def ph_l0():
                # ---------------- load inputs + trig tables ----------------
                mtab = S["mtab"] = pool.tile([128, S0, 3], f32, tag="mtab", name="mtab")
                nc.sync.dma_start(mtab[:], mtab_d[:])
                phi = pool.tile([128, S0, NQ], f32, tag="phi", name="phi")
                pih = pool.tile([128, 1], f32, tag="pih", name="pih")
                zero = pool.tile([128, 1], f32, tag="zero", name="zero")
                nc.vector.memset(pih[:], float(np.pi / 2))
                nc.vector.memset(zero[:], 0.0)
                cosT = pool.tile([128, S0, NQ], f32, tag="cosT", name="cosT")
                snsT = pool.tile([128, S0, 2, NQ], f32, tag="snsT", name="snsT")
                absT = pool.tile([128, S0, NQ], f32, tag="absT", name="absT")
                SCH = 4  # s-chunk for trig build (overlaps with L0 start)
                for ch in range(S0 // SCH):
                    sl = slice(ch * SCH, (ch + 1) * SCH)
                    nc.sync.dma_start(phi[:, sl, :], phi_d[:, sl, :])
                    nc.scalar.activation(absT[:, sl, :], phi[:, sl, :], Act.Abs, bias=zero[:, :])
                    nc.scalar.activation(cosT[:, sl, :], absT[:, sl, :], Act.Sin, bias=pih[:, :], scale=-1.0)
                    nc.scalar.activation(snsT[:, sl, 0, :], phi[:, sl, :], Act.Sin, bias=zero[:, :])
                    nc.scalar.activation(snsT[:, sl, 1, :], phi[:, sl, :], Act.Sin, bias=zero[:, :], scale=-1.0)

                # ---------------- L0 serial chain (fp32) ----------------
                # R[p, col k, row i, q] (col-outer so every slice is one
                # contiguous run); cols evolve under Rx (cols 1,2) and
                # Rz (cols 0,1); w_s = r * col0 extracted on the Act engine.
                R = S["R"] = pool.tile([128, 3, 3, NQ], f32, tag="R", name="R")
                w = S["w"] = pool.tile([128, 3, S0, NQ], f16, tag="w", name="w")
                m12 = pool.tile([128, 2, 3, NQ], f32, tag="m12", name="m12")
                m34 = pool.tile([128, 2, 3, NQ], f32, tag="m34", name="m34")
                tb01 = pool.tile([128, 2, 3, NQ], f32, tag="tb01", name="tb01")
                nc.vector.memset(R[:], 0.0)
                for i in range(3):
                    nc.vector.memset(R[:, i, i, :], 1.0)

                R12 = R[:, 1:3, :, :]
                R21 = R12[:, ::-1, :, :]
                R01 = R[:, 0:2, :, :]
                for s in range(S0):
                    cosb = cosT[:, s : s + 1, :].unsqueeze(2).broadcast_to([128, 2, 3, NQ])
                    snsb = snsT[:, s, :, :].unsqueeze(2).broadcast_to([128, 2, 3, NQ])
                    # Rx: (c1,c2) <- (cp*c1 + sp*c2, cp*c2 - sp*c1)
                    nc.vector.tensor_tensor(m12[:], R12, cosb, op=Al.mult)
                    nc.vector.tensor_tensor(m34[:], R21, snsb, op=Al.mult)
                    nc.vector.tensor_tensor(R12, m12[:], m34[:], op=Al.add)
                    # Rz: (c0,c1) <- (ct*c0 + st*c1, ct*c1 - st*c0)
                    # tb01 = [st*c0, st*c1]; consumers cross-index the slices
                    nc.vector.tensor_scalar(tb01[:], R01, mtab[:, s, 1:2], None, op0=Al.mult)
                    nc.vector.scalar_tensor_tensor(
                        R[:, 0], R[:, 0], mtab[:, s, 0:1], tb01[:, 1],
                        op0=Al.mult, op1=Al.add,
                    )
                    nc.vector.scalar_tensor_tensor(
                        R[:, 1], R[:, 1], mtab[:, s, 0:1], tb01[:, 0],
                        op0=Al.mult, op1=Al.subtract,
                    )
                    # w_s = r * col0(R)  (Act engine, fp16 out)
                    nc.scalar.mul(w[:, :, s, :], R[:, 0], mtab[:, s, 2:3])

            def ph_scan():
                # ------------- block-prefix rotations (fp16) -------------
                R = S["R"]
                Pq = pool.tile([128, NQ, 9], f16, tag="Pq", name="Pq")
                nc.vector.tensor_copy(
                    Pq[:].rearrange("p q (k i) -> p q k i", k=3),
                    R[:].rearrange("p k i q -> p q k i"),
                )
                ma = pool.tile([128, NQ // 2, 9], f16, tag="ma", name="ma")
                mb = pool.tile([128, NQ // 2, 9], f16, tag="mb", name="mb")
                mc = pool.tile([128, NQ // 2, 9], f16, tag="mc", name="mc")

                def compose(dst, aR, bR, g):
                    """dst = Ra @ Rb (column-major entries e = 3k + i)."""
                    for kk in range(3):
                        colk = aR[:, :, 3 * kk : 3 * kk + 3].unsqueeze(2).broadcast_to([128, g, 3, 3])
                        rowk = bR[:, :, kk::3].unsqueeze(3).broadcast_to([128, g, 3, 3])
                        mv = (ma, mb, mc)[kk][:, 0:g].rearrange("p g (f t) -> p g f t", f=3)
                        nc.vector.tensor_tensor(mv, colk, rowk, op=Al.mult)
                    nc.vector.tensor_tensor(ma[:, 0:g], ma[:, 0:g], mb[:, 0:g], op=Al.add)
                    nc.vector.tensor_tensor(dst, ma[:, 0:g], mc[:, 0:g], op=Al.add)

                # Brent-Kung in-place inclusive scan over q (per half)
                d = 1
                while d < NQ:
                    n = NQ // (2 * d)
                    a = Pq[:].rearrange("p (m j) e -> p m j e", j=2 * d)[:, :, d - 1, :]
                    b = Pq[:].rearrange("p (m j) e -> p m j e", j=2 * d)[:, :, 2 * d - 1, :]
                    compose(b, a, b, n)
                    d *= 2
                d = NQ // 4
                while d >= 1:
                    n = NQ // (2 * d) - 1
                    a = Pq[:].rearrange("p (m j) e -> p m j e", j=2 * d)[:, 0:n, 2 * d - 1, :]
                    b = Pq[:].rearrange("p (m j) e -> p m j e", j=2 * d)[:, 1 : n + 1, d - 1, :]
                    compose(b, a, b, n)
                    d //= 2

                # cross-half: lower-half total -> upper partitions
                stgRh = pool.tile([128, 1, 9], f16, tag="stgRh", name="stgRh")
                nc.sync.dma_start(stgRh[64:128, :, :], Pq[0:64, NQ - 1 : NQ, :])
                stgR = pool.tile([128, 1, 9], f32, tag="stgR", name="stgR")
                nc.vector.tensor_copy(stgR[64:128, :, :], stgRh[64:128, :, :])

                # ShR (exclusive prefix, matvec layout [p, i, k, q], fp16)
                ShR = S["ShR"] = pool.tile([128, 3, 3, NQ], f16, tag="ShR", name="ShR")
                nc.vector.memset(ShR[:, :, :, 0:1], 0.0)
                for i in range(3):
                    nc.vector.memset(ShR[0:64, i, i, 0:1], 1.0)
                nc.vector.tensor_copy(
                    ShR[0:64, :, :, 1:NQ],
                    Pq[0:64, 0 : NQ - 1, :].rearrange("p q (k i) -> p i k q", k=3),
                )
                # upper half: ShR_up[q] = stgR o Pshift_up[q]
                tmp = pool.tile([128, 3, 3, NQ], f16, tag="tmpU", name="tmpU")
                nc.vector.memset(tmp[64:128, :, :, 0:1], 0.0)
                for i in range(3):
                    nc.vector.memset(tmp[64:128, i, i, 0:1], 1.0)
                nc.vector.tensor_copy(
                    tmp[64:128, :, :, 1:NQ],
                    Pq[64:128, 0 : NQ - 1, :].rearrange("p q (k i) -> p i k q", k=3),
                )
                sa = pool.tile([128, 3, NQ], f16, tag="sa", name="sa")
                sb = pool.tile([128, 3, NQ], f16, tag="sb", name="sb")
                for i in range(3):
                    nc.vector.tensor_scalar(
                        sa[64:128], tmp[64:128, 0, :, :], stgR[64:128, 0, 3 * 0 + i : 3 * 0 + i + 1],
                        None, op0=Al.mult,
                    )
                    nc.vector.tensor_scalar(
                        sb[64:128], tmp[64:128, 1, :, :], stgR[64:128, 0, 3 * 1 + i : 3 * 1 + i + 1],
                        None, op0=Al.mult,
                    )
                    nc.vector.tensor_tensor(sa[64:128], sa[64:128], sb[64:128], op=Al.add)
                    nc.vector.tensor_scalar(
                        sb[64:128], tmp[64:128, 2, :, :], stgR[64:128, 0, 3 * 2 + i : 3 * 2 + i + 1],
                        None, op0=Al.mult,
                    )
                    nc.vector.tensor_tensor(ShR[64:128, i, :, :], sa[64:128], sb[64:128], op=Al.add)

            def ph_apply():
                # ---- v = ShR @ w ; cumsum in s; block-prefix; offset add ----
                w, ShR = S["w"], S["ShR"]
                v = S["v"] = pool.tile([128, 3, S0, NQ], f16, tag="v", name="v")
                u1 = pool.tile([128, S0, NQ], f16, tag="u1", name="u1")
                u2 = pool.tile([128, S0, NQ], f16, tag="u2", name="u2")
                for i in range(3):
                    sh = lambda j: ShR[:, i, j, :].unsqueeze(1).broadcast_to([128, S0, NQ])
                    nc.vector.tensor_tensor(v[:, i], w[:, 0], sh(0), op=Al.mult)
                    nc.vector.tensor_tensor(u1[:], w[:, 1], sh(1), op=Al.mult)
                    nc.vector.tensor_tensor(u2[:], w[:, 2], sh(2), op=Al.mult)
                    nc.vector.tensor_tensor(v[:, i], v[:, i], u1[:], op=Al.add)
                    nc.vector.tensor_tensor(v[:, i], v[:, i], u2[:], op=Al.add)
                # block-local cumsum over s (in place)
                for s in range(1, S0):
                    nc.vector.tensor_tensor(
                        v[:, :, s, :], v[:, :, s, :], v[:, :, s - 1, :], op=Al.add
                    )
                # block prefix: TI = inclusive scan_q of block totals (fp32)
                vz = pool.tile([128, NQ], f32, tag="vz", name="vz")
                nc.vector.memset(vz[:], 0.0)
                TI = pool.tile([128, 3, NQ], f32, tag="TI", name="TI")
                for i in range(3):
                    nc.vector.tensor_tensor_scan(
                        TI[:, i], v[:, i, S0 - 1, :], vz[:], 0.0, op0=Al.add, op1=Al.add
                    )
                # cross-half translation total
                stgT = pool.tile([128, 3, 1], f32, tag="stgT", name="stgT")
                nc.sync.dma_start(stgT[64:128], TI[0:64, :, NQ - 1 : NQ])
                Tpre = pool.tile([128, 3, NQ], f16, tag="Tpre", name="Tpre")
                nc.vector.memset(Tpre[:, :, 0:1], 0.0)
                nc.vector.tensor_copy(Tpre[0:64, :, 1:NQ], TI[0:64, :, 0 : NQ - 1])
                nc.vector.scalar_tensor_tensor(
                    Tpre[64:128, :, 1:NQ], TI[64:128, :, 0 : NQ - 1], 1.0,
                    stgT[64:128].broadcast_to([64, 3, NQ - 1]), op0=Al.mult, op1=Al.add,
                )
                nc.vector.tensor_copy(Tpre[64:128, :, 0:1], stgT[64:128])
                # coords = v + Tpre (broadcast over s) in place, fp16 2x
                nc.vector.tensor_tensor(
                    v[:], v[:], Tpre[:].unsqueeze(2).broadcast_to([128, 3, S0, NQ]),
                    op=Al.add,
                )

                # ---------------- transpose + store ----------------
                # Transpose tile (i, s) = [p, q] directly from the natural
                # layout; PSUM rows become the 128 blocks q of step s, and the
                # DMA's per-partition k-stride (24 rows) scatters them while
                # each partition still writes one contiguous 768B run.
                ident = pool.tile([128, 128], f16, tag="ident", name="ident")
                masks.make_identity(nc, ident[:])
                out_dv = out_d.rearrange("(h q t) c i -> h t q c i", h=NB2, q=NQ)
                with tc.tile_pool(name="psum", bufs=6, space="PSUM") as psum:
                    for s in range(S0):
                        for qh in range(NQ // 128):
                            ql = slice(qh * 128, (qh + 1) * 128)
                            Bk = pool.tile([128, NB2, BC, 3], f32, tag="Bk",
                                           name=f"Bk{s}_{qh}", bufs=2)
                            for i in range(3):
                                pt = psum.tile([128, 128], f16, tag="pt", name="pt")
                                nc.tensor.transpose(pt[:], v[:, i, s, ql], ident[:])
                                ptv = pt[:].rearrange("p (h c) -> p h c", h=NB2)
                                nc.scalar.copy(Bk[:, :, :, i], ptv)
                            for h in range(NB2):
                                nc.sync.dma_start(out_dv[h, s, ql], Bk[:, h])

            def dummy_out():
                nc.sync.dma_start(out_d[0:1, :, :], S["R"][0:1, 0, 0, 0:BC].unsqueeze(2).broadcast_to([1, BC, 3]))

            if only == "":
                for _rep in range(reps):
                    ph_l0(); ph_scan(); ph_apply()
            elif only == "l0":
                for _rep in range(reps):
                    ph_l0()
                dummy_out()
            elif only == "scan":
                ph_l0()
                for _rep in range(reps):
                    ph_scan()
                dummy_out()
            elif only == "l0scan":
                for _rep in range(reps):
                    ph_l0(); ph_scan()
                dummy_out()
            elif only == "apply":
                ph_l0(); ph_scan()
                for _rep in range(reps):
                    ph_apply()
            else:
                raise ValueError(only)

    nc.compile()
    return nc


def _get_program(reps: int = 1):
    import os as _os
    only = _os.environ.get("KM_ONLY", "")
    key = ("nc", reps, only)
    if key not in _CACHE:
        _CACHE[key] = _build_program(reps, only)
    return _CACHE[key]


def _make_mtab(core: int) -> np.ndarray:
    p = np.arange(128)
    c = p % 64
    bprime = 64 * core + c
    s = np.arange(S0)
    d = (2 * s[None, :] + bprime[:, None]) % 3
    mt = np.stack([_CT[d], _ST[d], np.broadcast_to(_BL[d], d.shape)], axis=-1)
    return np.ascontiguousarray(mt.astype(np.float32))


LAST_RUN = {}


def _get_runner(reps: int = 1):
    """Build (once) a cached jitted 8-core executable."""
    rkey = ("runner", reps)
    if rkey in _CACHE:
        return _CACHE[rkey]
    import jax
    from jax.sharding import Mesh, PartitionSpec
    from jax.experimental.shard_map import shard_map
    from concourse import bass2jax, mybir

    nc = _get_program(reps)
    bass2jax.install_neuronx_cc_hook()

    partition_name = nc.partition_id_tensor.name if nc.partition_id_tensor else None
    in_names, out_names, out_avals = [], [], []
    for alloc in nc.m.functions[0].allocations:
        if not isinstance(alloc, mybir.MemoryLocationSet):
            continue
        name = alloc.memorylocations[0].name
        if alloc.kind == "ExternalInput":
            if name != partition_name:
                in_names.append(name)
        elif alloc.kind == "ExternalOutput":
            out_names.append(name)
            out_avals.append(
                jax.core.ShapedArray(tuple(alloc.tensor_shape), mybir.dt.np(alloc.dtype))
            )
    n_params = len(in_names)
    all_names = list(in_names) + list(out_names)
    if partition_name is not None:
        all_names.append(partition_name)

    def _body(*args):
        operands = list(args)
        if partition_name is not None:
            operands.append(bass2jax.partition_id_tensor())
        outs = bass2jax._bass_exec_p.bind(
            *operands,
            out_avals=tuple(out_avals),
            in_names=tuple(all_names),
            out_names=tuple(out_names),
            lowering_input_output_aliases=(),
            sim_require_finite=True,
            sim_require_nnan=True,
            nc=nc,
        )
        return tuple(outs)

    devices = jax.devices()[:NUM_CORES]
    mesh = Mesh(np.asarray(devices), ("core",))
    nin = n_params + len(out_names)
    sharded = jax.jit(
        shard_map(
            _body,
            mesh=mesh,
            in_specs=(PartitionSpec("core"),) * nin,
            out_specs=(PartitionSpec("core"),) * len(out_names),
            check_rep=False,
        ),
        keep_unused=True,
    )
    _CACHE[rkey] = (sharded, in_names, out_names, mesh)
    return _CACHE[rkey]


def _prep_inputs(phi: np.ndarray):
    """phi [K, B_FULL] -> dict of concatenated per-core inputs.

    Device layout phi[p, s, q] with p = c + 64*h, global step
    k = 3072*h + 24*q + s, batch column b = 64*core + c."""
    cores = []
    phir = phi.reshape(NB2, NQ, S0, B_FULL)          # [h, q, s, b]
    for c in range(NUM_CORES):
        pc = phir[:, :, :, c * BC : (c + 1) * BC]    # [h, q, s, c]
        arr = pc.transpose(0, 3, 2, 1)               # [h, c, s, q]
        cores.append(np.ascontiguousarray(arr.reshape(128, S0, NQ)))
    phi_cat = np.concatenate(cores, axis=0)
    mtab_cat = np.concatenate([_make_mtab(c) for c in range(NUM_CORES)], axis=0)
    return {"phi": phi_cat, "mtab": mtab_cat}


def _run(inputs: dict) -> np.ndarray:
    sharded, in_names, out_names, _ = _get_runner()
    zeros = [np.zeros((NUM_CORES * K, BC, 3), np.float32)]
    out_arrs = sharded(*[inputs[n] for n in in_names], *zeros)
    out_cat = np.asarray(out_arrs[0])
    return out_cat


def kernel(flat_dihedrals: np.ndarray, batch_size) -> np.ndarray:
    B = int(batch_size)
    assert B == B_FULL and flat_dihedrals.shape == (K * B // 3, 3), (
        f"hardcoded for {(K * B_FULL // 3, 3)}, got {flat_dihedrals.shape}, B={B}"
    )
    phi = np.ascontiguousarray(np.asarray(flat_dihedrals, dtype=np.float32).reshape(K, B))
    out_cat = _run(_prep_inputs(phi))
    per_core = out_cat.reshape(NUM_CORES, K, BC, 3)
    out = np.empty((K, B, 3), dtype=np.float32)
    for core in range(NUM_CORES):
        out[:, core * BC : (core + 1) * BC, :] = per_core[core]
    return out


# revision 21
# speedup vs baseline: 1.0695x; 1.0199x over previous
"""Trainium2 Bass kernel for the NeRF coordinate-chain problem.

Reference semantics: flat_dihedrals [1048576, 3] is (row-major) reinterpreted
as phi[K=6144, B=512]; each of the 512 columns is an independent serial NeRF
chain of K rigid-body extension steps, with bond-geometry constants cycling as
d = (2k + b) mod 3.

Reformulation (v2): the per-step update is an affine (SE3) composition
    T_k = T_{k-1} o A_k,  A_k = [[Rx(phi_k) Rz(theta_d), p_k],[0,1]],
    p_k = r_d * col0(Rx Rz),  T_0 = Identity,  coord_k = translation(T_k).
Because rotations distribute over sums, coord_k = sum_{k'<=k} v_k' with
v_k' = Rprefix[k'-1] @ p_k' -- a pure cumulative sum of rotated step
increments.  Pipeline per core (64 batch columns, partitions p = c + 64*h
carrying two k-halves):
  L0 (fp32, DVE + Scalar): 24-step serial chain over NQ=128 blocks/half in a
    q-innermost layout R[p, row, col, q]; trig tables from the Act engine;
    w_s = r * col0(R) extracted on the Act engine into fp16.
  Block scan (fp16, DVE): Sklansky prefix over the 128 block rotations per
    half + cross-half fixup via a tiny SBUF DMA.
  Apply (fp16, DVE): v = ShR @ w as 9 fused-broadcast multiplies; block-local
    serial cumsum over s; per-block prefix via the hardware scan; broadcast
    block-offset add.
  Output: PE transposes (fp16 -> PSUM fp32 upconvert) into k-major staging
    tiles, contiguous DMAs to DRAM.

Sharding: batch columns split across 8 cores (64 columns/core).
"""

import numpy as np

L_STEPS = 2048
B_FULL = 512
NUM_CORES = 8
BC = B_FULL // NUM_CORES          # batch columns per core
K = 3 * L_STEPS                   # chain length = 6144
S0 = 12                           # L0 block size (multiple of 3)
NQ = 256                          # blocks per partition-half
NB2 = 2                           # partition halves (k = 3072*h + 24*q + s)
KTW = 96                          # k-tile width for output transposes (4 blocks)
NKT = (S0 * NQ) // KTW            # 32 k-tiles per half

_BL = np.array([145.801, 152.326, 132.868], dtype=np.float32)
_BA = np.array([2.124, 1.941, 2.028], dtype=np.float32)
_CT = np.cos(np.pi - _BA).astype(np.float32)
_ST = np.sin(np.pi - _BA).astype(np.float32)

_CACHE = {}


def _build_program(reps: int = 1, only: str = ""):
    """Build the program.  `only` repeats a single phase inside the reps loop
    ("l0" | "scan" | "apply") for phase-level HW timing; "" = full kernel
    repeated per rep."""
    import concourse.bass as bass
    import concourse.tile as tile
    from concourse import bacc, masks, mybir
    from concourse._compat import axon_active

    f32 = mybir.dt.float32
    f16 = mybir.dt.float16
    Al = mybir.AluOpType
    Act = mybir.ActivationFunctionType

    nc = bacc.Bacc(
        "TRN2",
        target_bir_lowering=False,
        debug=not axon_active(),
        enable_asserts=False,
        num_devices=NUM_CORES,
    )
    phi_d = nc.dram_tensor("phi", [128, S0, NQ], f32, kind="ExternalInput").ap()
    mtab_d = nc.dram_tensor("mtab", [128, S0, 3], f32, kind="ExternalInput").ap()
    out_d = nc.dram_tensor("out", [K, BC, 3], f32, kind="ExternalOutput").ap()

    with tile.TileContext(nc) as tc:
        with tc.tile_pool(name="main", bufs=1) as pool:
            S = {}

            def ph_l0():
                # ---------------- load inputs + trig tables ----------------
                mtab = S["mtab"] = pool.tile([128, S0, 3], f32, tag="mtab", name="mtab")
                nc.sync.dma_start(mtab[:], mtab_d[:])
                phi = pool.tile([128, S0, NQ], f32, tag="phi", name="phi")
                pih = pool.tile([128, 1], f32, tag="pih", name="pih")
                zero = pool.tile([128, 1], f32, tag="zero", name="zero")
                nc.vector.memset(pih[:], float(np.pi / 2))
                nc.vector.memset(zero[:], 0.0)
                cosT = pool.tile([128, S0, NQ], f32, tag="cosT", name="cosT")
                snsT = pool.tile([128, S0, 2, NQ], f32, tag="snsT", name="snsT")
                absT = pool.tile([128, S0, NQ], f32, tag="absT", name="absT")
                SCH = 4  # s-chunk for trig build (overlaps with L0 start)
                for ch in range(S0 // SCH):
                    sl = slice(ch * SCH, (ch + 1) * SCH)
                    nc.sync.dma_start(phi[:, sl, :], phi_d[:, sl, :])
                    nc.scalar.activation(absT[:, sl, :], phi[:, sl, :], Act.Abs, bias=zero[:, :])
                    nc.scalar.activation(cosT[:, sl, :], absT[:, sl, :], Act.Sin, bias=pih[:, :], scale=-1.0)
                    nc.scalar.activation(snsT[:, sl, 0, :], phi[:, sl, :], Act.Sin, bias=zero[:, :])
                    nc.scalar.activation(snsT[:, sl, 1, :], phi[:, sl, :], Act.Sin, bias=zero[:, :], scale=-1.0)

                # ---------------- L0 serial chain (fp32) ----------------
                # R[p, col k, row i, q] (col-outer so every slice is one
                # contiguous run); cols evolve under Rx (cols 1,2) and
                # Rz (cols 0,1); w_s = r * col0 extracted on the Act engine.
                R = S["R"] = pool.tile([128, 3, 3, NQ], f32, tag="R", name="R")
                w = S["w"] = pool.tile([128, 3, S0, NQ], f16, tag="w", name="w")
                m12 = pool.tile([128, 2, 3, NQ], f32, tag="m12", name="m12")
                m34 = pool.tile([128, 2, 3, NQ], f32, tag="m34", name="m34")
                tb01 = pool.tile([128, 2, 3, NQ], f32, tag="tb01", name="tb01")
                nc.vector.memset(R[:], 0.0)
                for i in range(3):
                    nc.vector.memset(R[:, i, i, :], 1.0)

                R12 = R[:, 1:3, :, :]
                R21 = R12[:, ::-1, :, :]
                R01 = R[:, 0:2, :, :]
                for s in range(S0):
                    cosb = cosT[:, s : s + 1, :].unsqueeze(2).broadcast_to([128, 2, 3, NQ])
                    snsb = snsT[:, s, :, :].unsqueeze(2).broadcast_to([128, 2, 3, NQ])
                    # Rx: (c1,c2) <- (cp*c1 + sp*c2, cp*c2 - sp*c1)
                    nc.vector.tensor_tensor(m12[:], R12, cosb, op=Al.mult)
                    nc.vector.tensor_tensor(m34[:], R21, snsb, op=Al.mult)
                    nc.vector.tensor_tensor(R12, m12[:], m34[:], op=Al.add)
                    # Rz: (c0,c1) <- (ct*c0 + st*c1, ct*c1 - st*c0)
                    # tb01 = [st*c0, st*c1]; consumers cross-index the slices
                    nc.vector.tensor_scalar(tb01[:], R01, mtab[:, s, 1:2], None, op0=Al.mult)
                    nc.vector.scalar_tensor_tensor(
                        R[:, 0], R[:, 0], mtab[:, s, 0:1], tb01[:, 1],
                        op0=Al.mult, op1=Al.add,
                    )
                    nc.vector.scalar_tensor_tensor(
                        R[:, 1], R[:, 1], mtab[:, s, 0:1], tb01[:, 0],
                        op0=Al.mult, op1=Al.subtract,
                    )
                    # w_s = r * col0(R)  (Act engine, fp16 out)
                    nc.scalar.mul(w[:, :, s, :], R[:, 0], mtab[:, s, 2:3])

            def ph_scan():
                # ------------- block-prefix rotations (fp16) -------------
                R = S["R"]
                Pq = pool.tile([128, NQ, 9], f16, tag="Pq", name="Pq")
                nc.vector.tensor_copy(
                    Pq[:].rearrange("p q (k i) -> p q k i", k=3),
                    R[:].rearrange("p k i q -> p q k i"),
                )
                ma = pool.tile([128, NQ // 2, 9], f16, tag="ma", name="ma")
                mb = pool.tile([128, NQ // 2, 9], f16, tag="mb", name="mb")
                mc = pool.tile([128, NQ // 2, 9], f16, tag="mc", name="mc")

                def compose(dst, aR, bR, g):
                    """dst = Ra @ Rb (column-major entries e = 3k + i)."""
                    for kk in range(3):
                        colk = aR[:, :, 3 * kk : 3 * kk + 3].unsqueeze(2).broadcast_to([128, g, 3, 3])
                        rowk = bR[:, :, kk::3].unsqueeze(3).broadcast_to([128, g, 3, 3])
                        mv = (ma, mb, mc)[kk][:, 0:g].rearrange("p g (f t) -> p g f t", f=3)
                        nc.vector.tensor_tensor(mv, colk, rowk, op=Al.mult)
                    nc.vector.tensor_tensor(ma[:, 0:g], ma[:, 0:g], mb[:, 0:g], op=Al.add)
                    nc.vector.tensor_tensor(dst, ma[:, 0:g], mc[:, 0:g], op=Al.add)

                # Brent-Kung in-place inclusive scan over q (per half)
                d = 1
                while d < NQ:
                    n = NQ // (2 * d)
                    a = Pq[:].rearrange("p (m j) e -> p m j e", j=2 * d)[:, :, d - 1, :]
                    b = Pq[:].rearrange("p (m j) e -> p m j e", j=2 * d)[:, :, 2 * d - 1, :]
                    compose(b, a, b, n)
                    d *= 2
                d = NQ // 4
                while d >= 1:
                    n = NQ // (2 * d) - 1
                    a = Pq[:].rearrange("p (m j) e -> p m j e", j=2 * d)[:, 0:n, 2 * d - 1, :]
                    b = Pq[:].rearrange("p (m j) e -> p m j e", j=2 * d)[:, 1 : n + 1, d - 1, :]
                    compose(b, a, b, n)
                    d //= 2

                # cross-half: lower-half total -> upper partitions
                stgRh = pool.tile([128, 1, 9], f16, tag="stgRh", name="stgRh")
                nc.sync.dma_start(stgRh[64:128, :, :], Pq[0:64, NQ - 1 : NQ, :])
                stgR = pool.tile([128, 1, 9], f32, tag="stgR", name="stgR")
                nc.vector.tensor_copy(stgR[64:128, :, :], stgRh[64:128, :, :])

                # ShR (exclusive prefix, matvec layout [p, i, k, q], fp16)
                ShR = S["ShR"] = pool.tile([128, 3, 3, NQ], f16, tag="ShR", name="ShR")
                nc.vector.memset(ShR[:, :, :, 0:1], 0.0)
                for i in range(3):
                    nc.vector.memset(ShR[0:64, i, i, 0:1], 1.0)
                nc.vector.tensor_copy(
                    ShR[0:64, :, :, 1:NQ],
                    Pq[0:64, 0 : NQ - 1, :].rearrange("p q (k i) -> p i k q", k=3),
                )
                # upper half: ShR_up[q] = stgR o Pshift_up[q]
                tmp = pool.tile([128, 3, 3, NQ], f16, tag="tmpU", name="tmpU")
                nc.vector.memset(tmp[64:128, :, :, 0:1], 0.0)
                for i in range(3):
                    nc.vector.memset(tmp[64:128, i, i, 0:1], 1.0)
                nc.vector.tensor_copy(
                    tmp[64:128, :, :, 1:NQ],
                    Pq[64:128, 0 : NQ - 1, :].rearrange("p q (k i) -> p i k q", k=3),
                )
                sa = pool.tile([128, 3, NQ], f16, tag="sa", name="sa")
                sb = pool.tile([128, 3, NQ], f16, tag="sb", name="sb")
                for i in range(3):
                    nc.vector.tensor_scalar(
                        sa[64:128], tmp[64:128, 0, :, :], stgR[64:128, 0, 3 * 0 + i : 3 * 0 + i + 1],
                        None, op0=Al.mult,
                    )
                    nc.vector.tensor_scalar(
                        sb[64:128], tmp[64:128, 1, :, :], stgR[64:128, 0, 3 * 1 + i : 3 * 1 + i + 1],
                        None, op0=Al.mult,
                    )
                    nc.vector.tensor_tensor(sa[64:128], sa[64:128], sb[64:128], op=Al.add)
                    nc.vector.tensor_scalar(
                        sb[64:128], tmp[64:128, 2, :, :], stgR[64:128, 0, 3 * 2 + i : 3 * 2 + i + 1],
                        None, op0=Al.mult,
                    )
                    nc.vector.tensor_tensor(ShR[64:128, i, :, :], sa[64:128], sb[64:128], op=Al.add)

            def ph_apply():
                # ---- v = ShR @ w ; cumsum in s; block-prefix; offset add ----
                w, ShR = S["w"], S["ShR"]
                v = S["v"] = pool.tile([128, 3, S0, NQ], f16, tag="v", name="v")
                u1 = pool.tile([128, S0, NQ], f16, tag="u1", name="u1")
                u2 = pool.tile([128, S0, NQ], f16, tag="u2", name="u2")
                for i in range(3):
                    sh = lambda j: ShR[:, i, j, :].unsqueeze(1).broadcast_to([128, S0, NQ])
                    nc.vector.tensor_tensor(v[:, i], w[:, 0], sh(0), op=Al.mult)
                    nc.vector.tensor_tensor(u1[:], w[:, 1], sh(1), op=Al.mult)
                    nc.vector.tensor_tensor(u2[:], w[:, 2], sh(2), op=Al.mult)
                    nc.vector.tensor_tensor(v[:, i], v[:, i], u1[:], op=Al.add)
                    nc.vector.tensor_tensor(v[:, i], v[:, i], u2[:], op=Al.add)
                # block-local cumsum over s (in place)
                for s in range(1, S0):
                    nc.vector.tensor_tensor(
                        v[:, :, s, :], v[:, :, s, :], v[:, :, s - 1, :], op=Al.add
                    )
                # block prefix: TI = inclusive scan_q of block totals (fp32)
                vz = pool.tile([128, NQ], f32, tag="vz", name="vz")
                nc.vector.memset(vz[:], 0.0)
                TI = pool.tile([128, 3, NQ], f32, tag="TI", name="TI")
                for i in range(3):
                    nc.vector.tensor_tensor_scan(
                        TI[:, i], v[:, i, S0 - 1, :], vz[:], 0.0, op0=Al.add, op1=Al.add
                    )
                # cross-half translation total
                stgT = pool.tile([128, 3, 1], f32, tag="stgT", name="stgT")
                nc.sync.dma_start(stgT[64:128], TI[0:64, :, NQ - 1 : NQ])
                Tpre = pool.tile([128, 3, NQ], f16, tag="Tpre", name="Tpre")
                nc.vector.memset(Tpre[:, :, 0:1], 0.0)
                nc.vector.tensor_copy(Tpre[0:64, :, 1:NQ], TI[0:64, :, 0 : NQ - 1])
                nc.vector.scalar_tensor_tensor(
                    Tpre[64:128, :, 1:NQ], TI[64:128, :, 0 : NQ - 1], 1.0,
                    stgT[64:128].broadcast_to([64, 3, NQ - 1]), op0=Al.mult, op1=Al.add,
                )
                nc.vector.tensor_copy(Tpre[64:128, :, 0:1], stgT[64:128])
                # coords = v + Tpre (broadcast over s) in place, fp16 2x
                nc.vector.tensor_tensor(
                    v[:], v[:], Tpre[:].unsqueeze(2).broadcast_to([128, 3, S0, NQ]),
                    op=Al.add,
                )

                # ---------------- transpose + store ----------------
                # Transpose tile (i, s) = [p, q] directly from the natural
                # layout; PSUM rows become the 128 blocks q of step s, and the
                # DMA's per-partition k-stride (24 rows) scatters them while
                # each partition still writes one contiguous 768B run.
                ident = pool.tile([128, 128], f16, tag="ident", name="ident")
                masks.make_identity(nc, ident[:])
                out_dv = out_d.rearrange("(h q t) c i -> h t q c i", h=NB2, q=NQ)
                with tc.tile_pool(name="psum", bufs=6, space="PSUM") as psum:
                    for s in range(S0):
                        for qh in range(NQ // 128):
                            ql = slice(qh * 128, (qh + 1) * 128)
                            Bk = pool.tile([128, NB2, BC, 3], f32, tag="Bk",
                                           name=f"Bk{s}_{qh}", bufs=2)
                            for i in range(3):
                                pt = psum.tile([128, 128], f16, tag="pt", name="pt")
                                nc.tensor.transpose(pt[:], v[:, i, s, ql], ident[:])
                                ptv = pt[:].rearrange("p (h c) -> p h c", h=NB2)
                                if i == 0:
                                    nc.vector.tensor_copy(Bk[:, :, :, 0], ptv)
                                else:
                                    nc.scalar.copy(Bk[:, :, :, i], ptv)
                            for h in range(NB2):
                                nc.sync.dma_start(out_dv[h, s, ql], Bk[:, h])

            def dummy_out():
                nc.sync.dma_start(out_d[0:1, :, :], S["R"][0:1, 0, 0, 0:BC].unsqueeze(2).broadcast_to([1, BC, 3]))

            if only == "":
                for _rep in range(reps):
                    ph_l0(); ph_scan(); ph_apply()
            elif only == "l0":
                for _rep in range(reps):
                    ph_l0()
                dummy_out()
            elif only == "scan":
                ph_l0()
                for _rep in range(reps):
                    ph_scan()
                dummy_out()
            elif only == "l0scan":
                for _rep in range(reps):
                    ph_l0(); ph_scan()
                dummy_out()
            elif only == "apply":
                ph_l0(); ph_scan()
                for _rep in range(reps):
                    ph_apply()
            else:
                raise ValueError(only)

    nc.compile()
    return nc


def _get_program(reps: int = 1):
    import os as _os
    only = _os.environ.get("KM_ONLY", "")
    key = ("nc", reps, only)
    if key not in _CACHE:
        _CACHE[key] = _build_program(reps, only)
    return _CACHE[key]


def _make_mtab(core: int) -> np.ndarray:
    p = np.arange(128)
    c = p % 64
    bprime = 64 * core + c
    s = np.arange(S0)
    d = (2 * s[None, :] + bprime[:, None]) % 3
    mt = np.stack([_CT[d], _ST[d], np.broadcast_to(_BL[d], d.shape)], axis=-1)
    return np.ascontiguousarray(mt.astype(np.float32))


LAST_RUN = {}


def _get_runner(reps: int = 1):
    """Build (once) a cached jitted 8-core executable."""
    rkey = ("runner", reps)
    if rkey in _CACHE:
        return _CACHE[rkey]
    import jax
    from jax.sharding import Mesh, PartitionSpec
    from jax.experimental.shard_map import shard_map
    from concourse import bass2jax, mybir

    nc = _get_program(reps)
    bass2jax.install_neuronx_cc_hook()

    partition_name = nc.partition_id_tensor.name if nc.partition_id_tensor else None
    in_names, out_names, out_avals = [], [], []
    for alloc in nc.m.functions[0].allocations:
        if not isinstance(alloc, mybir.MemoryLocationSet):
            continue
        name = alloc.memorylocations[0].name
        if alloc.kind == "ExternalInput":
            if name != partition_name:
                in_names.append(name)
        elif alloc.kind == "ExternalOutput":
            out_names.append(name)
            out_avals.append(
                jax.core.ShapedArray(tuple(alloc.tensor_shape), mybir.dt.np(alloc.dtype))
            )
    n_params = len(in_names)
    all_names = list(in_names) + list(out_names)
    if partition_name is not None:
        all_names.append(partition_name)

    def _body(*args):
        operands = list(args)
        if partition_name is not None:
            operands.append(bass2jax.partition_id_tensor())
        outs = bass2jax._bass_exec_p.bind(
            *operands,
            out_avals=tuple(out_avals),
            in_names=tuple(all_names),
            out_names=tuple(out_names),
            lowering_input_output_aliases=(),
            sim_require_finite=True,
            sim_require_nnan=True,
            nc=nc,
        )
        return tuple(outs)

    devices = jax.devices()[:NUM_CORES]
    mesh = Mesh(np.asarray(devices), ("core",))
    nin = n_params + len(out_names)
    sharded = jax.jit(
        shard_map(
            _body,
            mesh=mesh,
            in_specs=(PartitionSpec("core"),) * nin,
            out_specs=(PartitionSpec("core"),) * len(out_names),
            check_rep=False,
        ),
        keep_unused=True,
    )
    _CACHE[rkey] = (sharded, in_names, out_names, mesh)
    return _CACHE[rkey]


def _prep_inputs(phi: np.ndarray):
    """phi [K, B_FULL] -> dict of concatenated per-core inputs.

    Device layout phi[p, s, q] with p = c + 64*h, global step
    k = 3072*h + 24*q + s, batch column b = 64*core + c."""
    cores = []
    phir = phi.reshape(NB2, NQ, S0, B_FULL)          # [h, q, s, b]
    for c in range(NUM_CORES):
        pc = phir[:, :, :, c * BC : (c + 1) * BC]    # [h, q, s, c]
        arr = pc.transpose(0, 3, 2, 1)               # [h, c, s, q]
        cores.append(np.ascontiguousarray(arr.reshape(128, S0, NQ)))
    phi_cat = np.concatenate(cores, axis=0)
    mtab_cat = np.concatenate([_make_mtab(c) for c in range(NUM_CORES)], axis=0)
    return {"phi": phi_cat, "mtab": mtab_cat}


def _run(inputs: dict) -> np.ndarray:
    sharded, in_names, out_names, _ = _get_runner()
    zeros = [np.zeros((NUM_CORES * K, BC, 3), np.float32)]
    out_arrs = sharded(*[inputs[n] for n in in_names], *zeros)
    out_cat = np.asarray(out_arrs[0])
    return out_cat


def kernel(flat_dihedrals: np.ndarray, batch_size) -> np.ndarray:
    B = int(batch_size)
    assert B == B_FULL and flat_dihedrals.shape == (K * B // 3, 3), (
        f"hardcoded for {(K * B_FULL // 3, 3)}, got {flat_dihedrals.shape}, B={B}"
    )
    phi = np.ascontiguousarray(np.asarray(flat_dihedrals, dtype=np.float32).reshape(K, B))
    out_cat = _run(_prep_inputs(phi))
    per_core = out_cat.reshape(NUM_CORES, K, BC, 3)
    out = np.empty((K, B, 3), dtype=np.float32)
    for core in range(NUM_CORES):
        out[:, core * BC : (core + 1) * BC, :] = per_core[core]
    return out
